# revision 1
# baseline (speedup 1.0000x reference)
"""Trainium2 Bass kernel for Transformer-XL relative attention (nn_Attention).

Sharding: 8 cores = data-parallel over batch (2) x tensor-parallel over heads
(16 -> 4 per core).  Each core computes its 4 heads' attention for its batch,
a partial output projection, then ReduceScatter(add) over its batch quad;
each core LayerNorms its 512-row output shard.

Device-side structure (per core):
- fp32r matmuls (full PE rate at free-dim >= 256).
- The reference's _rel_shift (shear with cross-row wraparound) is computed
  exactly via a flat DRAM buffer: bd rows written at stride L+1 with a
  leading zero; rows of length L re-read from offset L give the shifted
  matrix.  Contiguous bf16 DMA both directions.
- scores(nat) = ac matmul (K=64) + identity-matmul add of shifted bd (bf16).
- scores -> bf16 -> PE transpose-mode -> bf16 PSUM -> exp on ACT gives P^T
  in SBUF for the context matmul.
- mask + softmax denominator ride the context matmul: V+ = [V*mask | mask],
  so psum row 64 is the masked denominator (masked cols contribute exact 0,
  = reference's exp(-inf)).
"""

import numpy as np

B, L, D, NH, DH = 2, 2048, 1024, 16, 64
NHL = 4
P = 128
SCALE = 1.0 / np.sqrt(DH)
LN_EPS = 1e-5
N_CORES = 8

_CACHE = {}


def _build_program():
    import concourse.bacc as bacc
    import concourse.mybir as mybir
    import concourse.tile as tile
    from concourse.masks import make_identity

    F32 = mybir.dt.float32
    F16 = mybir.dt.float16
    AF = mybir.ActivationFunctionType
    AX = mybir.AxisListType
    OP = mybir.AluOpType

    nc = bacc.Bacc("TRN2", target_bir_lowering=False, debug=False,
                   num_devices=N_CORES)

    xT = nc.declare_dram_parameter("xT", [D, L], F16, isOutput=False)
    relT = nc.declare_dram_parameter("relT", [D, L], F16, isOutput=False)
    xres = nc.declare_dram_parameter("xres", [512, D], F32, isOutput=False)
    Wq = nc.declare_dram_parameter("Wq", [D, 256], F16, isOutput=False)
    Wk = nc.declare_dram_parameter("Wk", [D, 256], F16, isOutput=False)
    Wv = nc.declare_dram_parameter("Wv", [D, 256], F16, isOutput=False)
    Wrel = nc.declare_dram_parameter("Wrel", [D, 256], F16, isOutput=False)
    Wout = nc.declare_dram_parameter("Wout", [256, D], F16, isOutput=False)
    rwb = nc.declare_dram_parameter("rwb", [256], F32, isOutput=False)
    rrb = nc.declare_dram_parameter("rrb", [256], F32, isOutput=False)
    mask01 = nc.declare_dram_parameter("mask01", [L], F32, isOutput=False)
    gamma = nc.declare_dram_parameter("gamma", [D], F32, isOutput=False)
    beta = nc.declare_dram_parameter("beta", [D], F32, isOutput=False)
    out = nc.declare_dram_parameter("out", [512, D], F32, isOutput=True)

    with tile.TileContext(nc) as tc:
        with (
            tc.tile_pool(name="persist", bufs=1) as pers,
            tc.tile_pool(name="dram", bufs=1, space="DRAM") as dram,
        ):
            ident = pers.tile([P, P], F16)
            make_identity(nc, ident[:])
            ones_r = pers.tile([1, 64], F16)
            nc.vector.memset(ones_r[:], 1.0)
            nbias = pers.tile([P, 1], F32)
            nc.vector.memset(nbias[:], -4.0)
            m01 = pers.tile([P, 16], F32)
            nc.sync.dma_start(m01[:], mask01.rearrange("(o p) -> p o", p=P))

            rwT = [pers.tile([P, L], F16, name=f"rwT{c}") for c in range(2)]
            rrT = [pers.tile([P, L], F16, name=f"rrT{c}") for c in range(2)]
            kT = [pers.tile([P, L], F16, name=f"kT{c}") for c in range(2)]
            rkT = [pers.tile([P, L], F16, name=f"rkT{c}") for c in range(2)]
            vp = [pers.tile([P, 16, DH + 1], F16, name=f"vp{h}") for h in range(NHL)]
            ctxT = [pers.tile([P, L], F16, name=f"ctxT{c}") for c in range(2)]

            # ---------- Phase A: projections ----------
            with (
                tc.tile_pool(name="slab", bufs=2) as slab_p,
                tc.tile_pool(name="s32", bufs=1) as s32_p,
                tc.tile_pool(name="wr", bufs=1) as wr_p,
                tc.tile_pool(name="psA", bufs=2, space="PSUM") as psA,
            ):
                wq_r = wr_p.tile([P, 8, 256], F16)
                wk_r = wr_p.tile([P, 8, 256], F16)
                wv_r = wr_p.tile([P, 8, 256], F16)
                wl_r = wr_p.tile([P, 8, 256], F16)
                for wdram, wr in ((Wq, wq_r), (Wk, wk_r), (Wv, wv_r), (Wrel, wl_r)):
                    nc.sync.dma_start(wr[:], wdram.rearrange("(k p) n -> p k n", p=P))

                rwb_sb = wr_p.tile([P, 2], F32)
                nc.sync.dma_start(rwb_sb[:], rwb.rearrange("(c p) -> p c", p=P))
                rrb_sb = wr_p.tile([P, 2], F32)
                nc.sync.dma_start(rrb_sb[:], rrb.rearrange("(c p) -> p c", p=P))

                for ic in range(8):
                    I0 = 256 * ic
                    xs = slab_p.tile([P, 8, 256], F16, tag="xs")
                    nc.sync.dma_start(
                        xs[:], xT[:, I0:I0 + 256].rearrange("(k p) n -> p k n", p=P))
                    rsl = slab_p.tile([P, 8, 256], F16, tag="rsl")
                    nc.sync.dma_start(
                        rsl[:], relT[:, I0:I0 + 256].rearrange("(k p) n -> p k n", p=P))

                    for cc in range(2):
                        pq = psA.tile([P, 256], F32, tag="pA")
                        for k in range(8):
                            nc.tensor.matmul(pq[:], wq_r[:, k, 128 * cc:128 * cc + 128],
                                             xs[:, k, :], start=(k == 0), stop=(k == 7))
                        nc.vector.tensor_scalar_add(rwT[cc][:, I0:I0 + 256], pq[:],
                                                    rwb_sb[:, cc:cc + 1])
                        nc.vector.tensor_scalar_add(rrT[cc][:, I0:I0 + 256], pq[:],
                                                    rrb_sb[:, cc:cc + 1])
                        pk = psA.tile([P, 256], F32, tag="pA")
                        for k in range(8):
                            nc.tensor.matmul(pk[:], wk_r[:, k, 128 * cc:128 * cc + 128],
                                             xs[:, k, :], start=(k == 0), stop=(k == 7))
                        nc.scalar.copy(kT[cc][:, I0:I0 + 256], pk[:])
                        pr = psA.tile([P, 256], F32, tag="pA")
                        for k in range(8):
                            nc.tensor.matmul(pr[:], wl_r[:, k, 128 * cc:128 * cc + 128],
                                             rsl[:, k, :], start=(k == 0), stop=(k == 7))
                        nc.scalar.copy(rkT[cc][:, I0:I0 + 256], pr[:])

                    for jj in range(2):
                        jo = 2 * ic + jj
                        pv = psA.tile([P, 256], F32, tag="pA")
                        for k in range(8):
                            nc.tensor.matmul(pv[:], xs[:, k, 128 * jj:128 * jj + 128],
                                             wv_r[:, k, :], start=(k == 0), stop=(k == 7))
                        for h in range(NHL):
                            nc.vector.tensor_scalar_mul(
                                vp[h][:, jo, 0:DH], pv[:, DH * h:DH * h + DH],
                                m01[:, jo:jo + 1])
                            nc.vector.tensor_copy(vp[h][:, jo, DH:DH + 1],
                                                  m01[:, jo:jo + 1])

            # ---------- Phase B: attention ----------
            pf_bufs = [dram.tile([L * (L + 1)], F16, name=f"pf{i}") for i in range(2)]

            with (
                tc.tile_pool(name="wt", bufs=3) as wt_p,
                tc.tile_pool(name="sh", bufs=3) as sh_p,
                tc.tile_pool(name="s16", bufs=9) as s16_p,
                tc.tile_pool(name="pt", bufs=3) as pt_p,
                tc.tile_pool(name="bc", bufs=2) as bc_p,
                tc.tile_pool(name="oddt", bufs=2) as odd_p,
                tc.tile_pool(name="den", bufs=2) as den_p,
                tc.tile_pool(name="cs", bufs=2) as cs_p,
                tc.tile_pool(name="psB", bufs=1, space="PSUM") as psB,
                tc.tile_pool(name="psS", bufs=3, space="PSUM") as psS,
                tc.tile_pool(name="psT", bufs=2, space="PSUM") as psT,
                tc.tile_pool(name="psBc", bufs=2, space="PSUM") as psBc,
            ):

                for h in range(NHL):
                    pf = pf_bufs[h % 2][:]
                    cc, par = h // 2, h % 2
                    sA = slice(64 * par, 64 * par + 64)
                    pf2d = pf[0:L * (L + 1)].rearrange("(r c) -> r c", c=L + 1)

                    for ic in range(16):
                        I0 = 128 * ic
                        wt = wt_p.tile([P, 2049], F16, tag="wt")
                        nc.vector.memset(wt[:, 0:1], 0.0)
                        for t in range(4):
                            pbd = psB.tile([P, 512], F32, tag="bd")
                            nc.tensor.matmul(pbd[:],
                                             rrT[cc][sA, I0:I0 + 128],
                                             rkT[cc][sA, 512 * t:512 * t + 512],
                                             start=True, stop=True)
                            if t % 2 == 0:
                                nc.vector.tensor_copy(
                                    wt[:, 1 + 512 * t: 1 + 512 * t + 512], pbd[:])
                            else:
                                nc.scalar.copy(
                                    wt[:, 1 + 512 * t: 1 + 512 * t + 512], pbd[:])
                        nc.gpsimd.dma_start(pf2d[I0:I0 + 128, :], wt[:])

                    ot = odd_p.tile([64, L], F16, tag="odd", name="ot") if par == 1 else None

                    for half in range(2):
                        H0 = 1024 * half
                        s16s = []
                        for icc in range(8):
                            I0 = H0 + 128 * icc
                            sh16 = sh_p.tile([P, L], F16, tag="sh")
                            nc.gpsimd.dma_start(
                                sh16[:],
                                pf[L + I0 * L: L + (I0 + 128) * L]
                                .rearrange("(r c) -> r c", c=L))
                            s16 = s16_p.tile([P, L], F16, tag="s16")
                            for t in range(4):
                                psc = psS.tile([P, 512], F32, tag="sc")
                                nc.tensor.matmul(psc[:],
                                                 rwT[cc][sA, I0:I0 + 128],
                                                 kT[cc][sA, 512 * t:512 * t + 512],
                                                 start=True, stop=False)
                                nc.tensor.matmul(psc[:], ident[:],
                                                 sh16[:, 512 * t:512 * t + 512],
                                                 start=False, stop=True)
                                if t % 2 == 1:
                                    nc.vector.tensor_copy(
                                        s16[:, 512 * t:512 * t + 512], psc[:])
                                else:
                                    nc.scalar.copy(
                                        s16[:, 512 * t:512 * t + 512], psc[:])
                            s16s.append(s16)

                        pc0 = psBc.tile([65, 512], F32, tag="c")
                        pc1 = psBc.tile([65, 512], F32, tag="c")
                        for J in range(16):
                            ptp = psT.tile([P, 1024], F16, tag="pt")
                            for icc in range(8):
                                nc.tensor.matmul(ptp[:, 128 * icc:128 * icc + 128],
                                                 s16s[icc][:, 128 * J:128 * J + 128],
                                                 ident[:], is_transpose=True,
                                                 start=True, stop=True)
                            pt_sb = pt_p.tile([P, 1024], F16, tag="ptsb")
                            nc.scalar.activation(pt_sb[:], ptp[:], AF.Exp, bias=nbias[:])
                            for ii, pc in enumerate((pc0, pc1)):
                                nc.tensor.matmul(pc[:],
                                                 vp[h][:, J, :],
                                                 pt_sb[:, 512 * ii:512 * ii + 512],
                                                 start=(J == 0), stop=(J == 15))

                        cs0 = cs_p.tile([65, 512], F32, tag="cs0", name="cs0")
                        cs1 = cs_p.tile([65, 512], F32, tag="cs1", name="cs1")
                        nc.vector.tensor_copy(cs0[:], pc0[:])
                        nc.vector.tensor_copy(cs1[:], pc1[:])
                        den_sb = den_p.tile([P, 1024], F32, tag="den_sb", name="den_sb")
                        den0 = den_p.tile([1, 1024], F32, tag="den0", name="den0")
                        rec0 = den_p.tile([1, 1024], F32, tag="rec0", name="rec0")
                        recr = den_p.tile([1, 1024], F16, tag="recr", name="recr")
                        scr = den_p.tile([1, 1024], F32, tag="scr", name="scr")
                        nc.vector.tensor_copy(den_sb[64:65, 0:512], cs0[64:65, :])
                        nc.vector.tensor_copy(den_sb[64:65, 512:1024], cs1[64:65, :])
                        nc.sync.dma_start(den0[0:1, :], den_sb[64:65, 0:1024])
                        nc.vector.reciprocal_approx_accurate(
                            rec0[0:1, :], den0[0:1, :], scr[0:1, :])
                        nc.vector.tensor_copy(recr[0:1, :], rec0[0:1, :])
                        for ii, pc in enumerate((cs0, cs1)):
                            i0 = H0 + 512 * ii
                            pb = psS.tile([P, 512], F32, tag="sc")
                            nc.tensor.matmul(pb[0:64, :], ones_r[0:1, :],
                                             recr[0:1, 512 * ii:512 * ii + 512],
                                             start=True, stop=True)
                            bc = bc_p.tile([64, 512], F32, tag="bc")
                            nc.scalar.copy(bc[:], pb[0:64, :])
                            if par == 0:
                                nc.vector.tensor_mul(ctxT[cc][0:64, i0:i0 + 512],
                                                     pc[0:64, :], bc[:])
                            else:
                                nc.vector.tensor_mul(ot[:, i0:i0 + 512],
                                                     pc[0:64, :], bc[:])
                    if par == 1:
                        nc.sync.dma_start(ctxT[cc][64:128, :], ot[:, :])

            # ---------- Phase C: out projection + ReduceScatter + LayerNorm ----
            attn_d = dram.tile([L, D], F16)
            rs_d = dram.tile([512, D], F16)

            with (
                tc.tile_pool(name="wo", bufs=1) as wo_p,
                tc.tile_pool(name="oc", bufs=3) as oc_p,
                tc.tile_pool(name="psC", bufs=2, space="PSUM") as psC,
            ):
                wo_r = [wo_p.tile([P, 2, 512], F16, name=f"wo{c}") for c in range(2)]
                for c in range(2):
                    nc.sync.dma_start(
                        wo_r[c][:], Wout[128 * c:128 * c + 128, :]
                        .rearrange("p (t n) -> p t n", t=2))

                for r in range(4):
                    for ic in range(4 * r, 4 * r + 4):
                        I0 = 128 * ic
                        for t in range(2):
                            po = psC.tile([P, 512], F32, tag="po")
                            for c in range(2):
                                nc.tensor.matmul(po[:], ctxT[c][:, I0:I0 + 128],
                                                 wo_r[c][:, t, :],
                                                 start=(c == 0), stop=(c == 1))
                            ao = oc_p.tile([P, 512], F16, tag="ao")
                            if t == 0:
                                nc.vector.tensor_copy(ao[:], po[:])
                            else:
                                nc.scalar.copy(ao[:], po[:])
                            nc.sync.dma_start(
                                attn_d[I0:I0 + 128, 512 * t:512 * t + 512], ao[:])
                    nc.gpsimd.collective_compute(
                        "ReduceScatter", OP.add,
                        replica_groups=[[0, 1, 2, 3], [4, 5, 6, 7]],
                        ins=[attn_d[512 * r:512 * r + 512, :].opt()],
                        outs=[rs_d[128 * r:128 * r + 128, :].opt()],
                    )

            with (
                tc.tile_pool(name="ln", bufs=2) as ln_p,
                tc.tile_pool(name="lng", bufs=1) as lng_p,
            ):
                gb = lng_p.tile([P, D], F32)
                nc.gpsimd.dma_start(gb[:], gamma.ap().rearrange("(o d) -> o d", o=1).to_broadcast((P, D)))
                bb = lng_p.tile([P, D], F32)
                nc.gpsimd.dma_start(bb[:], beta.ap().rearrange("(o d) -> o d", o=1).to_broadcast((P, D)))

                for rc in range(4):
                    R0 = 128 * rc
                    zt16 = ln_p.tile([P, D], F16, tag="zt16")
                    nc.sync.dma_start(zt16[:], rs_d[R0:R0 + 128, :])
                    zt = ln_p.tile([P, D], F32, tag="zt")
                    nc.vector.tensor_copy(zt[:], zt16[:])
                    xr = ln_p.tile([P, D], F32, tag="xr")
                    nc.sync.dma_start(xr[:], xres[R0:R0 + 128, :])
                    nc.vector.tensor_add(zt[:], zt[:], xr[:])
                    mu = ln_p.tile([P, 1], F32, tag="mu")
                    nc.vector.tensor_reduce(mu[:], zt[:], AX.X, OP.add)
                    nc.vector.tensor_scalar_mul(mu[:], mu[:], 1.0 / D)
                    xc = ln_p.tile([P, D], F32, tag="xc")
                    nc.vector.tensor_scalar_sub(xc[:], zt[:], mu[:])
                    sq = ln_p.tile([P, D], F32, tag="sq")
                    nc.vector.tensor_mul(sq[:], xc[:], xc[:])
                    var = ln_p.tile([P, 1], F32, tag="var")
                    nc.vector.tensor_reduce(var[:], sq[:], AX.X, OP.add)
                    nc.vector.tensor_scalar_mul(var[:], var[:], 1.0 / D)
                    nc.vector.tensor_scalar_add(var[:], var[:], LN_EPS)
                    sd = ln_p.tile([P, 1], F32, tag="sd")
                    nc.scalar.activation(sd[:], var[:], AF.Sqrt)
                    isd = ln_p.tile([P, 1], F32, tag="isd")
                    nc.vector.reciprocal(isd[:], sd[:])
                    nc.vector.tensor_scalar_mul(xc[:], xc[:], isd[:])
                    nc.vector.tensor_mul(xc[:], xc[:], gb[:])
                    nc.vector.tensor_add(xc[:], xc[:], bb[:])
                    nc.sync.dma_start(out[R0:R0 + 128, :], xc[:])

    nc.compile()
    return nc


def _prep_inputs(x, relative_pos, r_w_bias, r_r_bias, attn_mask,
                 W_qkv, W_rel, W_out, ln_gamma, ln_beta):
    in_maps = []
    relT = np.ascontiguousarray(relative_pos.T).astype(np.float16)
    m01f = (~np.asarray(attn_mask).astype(bool)).astype(np.float32)
    for c in range(N_CORES):
        b, g = c // 4, c % 4
        h0 = 4 * g
        cols = slice(DH * h0, DH * h0 + 256)
        im = dict(
            xT=np.ascontiguousarray(x[b].T).astype(np.float16),
            relT=relT,
            xres=np.ascontiguousarray(np.concatenate(
                [x[b, 512 * r + 128 * g: 512 * r + 128 * g + 128, :]
                 for r in range(4)], axis=0)).astype(np.float32),
            Wq=np.ascontiguousarray(
                W_qkv[:, DH * h0:DH * h0 + 256] * SCALE).astype(np.float16),
            Wk=np.ascontiguousarray(
                W_qkv[:, D + DH * h0: D + DH * h0 + 256]).astype(np.float16),
            Wv=np.ascontiguousarray(
                W_qkv[:, 2 * D + DH * h0: 2 * D + DH * h0 + 256]).astype(np.float16),
            Wrel=np.ascontiguousarray(W_rel[:, cols]).astype(np.float16),
            Wout=np.ascontiguousarray(W_out[cols, :]).astype(np.float16),
            rwb=np.ascontiguousarray(
                r_w_bias[h0:h0 + 4].reshape(-1) * SCALE).astype(np.float32),
            rrb=np.ascontiguousarray(
                r_r_bias[h0:h0 + 4].reshape(-1) * SCALE).astype(np.float32),
            mask01=m01f[b],
            gamma=np.asarray(ln_gamma).astype(np.float32),
            beta=np.asarray(ln_beta).astype(np.float32),
        )
        in_maps.append(im)
    return in_maps


def kernel(**inputs):
    from concourse.bass_utils import run_bass_kernel_spmd

    if "nc" not in _CACHE:
        _CACHE["nc"] = _build_program()
    nc = _CACHE["nc"]

    in_maps = _prep_inputs(**{k: np.asarray(v) for k, v in inputs.items()})
    res = run_bass_kernel_spmd(nc, in_maps, list(range(N_CORES)))
    outp = np.empty((B, L, D), np.float32)
    for c in range(N_CORES):
        b, g = c // 4, c % 4
        o = res.results[c]["out"]
        for r in range(4):
            outp[b, 512 * r + 128 * g: 512 * r + 128 * g + 128, :] = \
                o[128 * r:128 * r + 128, :]
    return outp



# revision 14
# speedup vs baseline: 1.0044x; 1.0044x over previous
"""Trainium2 Bass kernel for Transformer-XL relative attention (nn_Attention).

Sharding: 8 cores = data-parallel over batch (2) x tensor-parallel over heads
(16 -> 4 per core).  Each core computes its 4 heads' attention for its batch,
a partial output projection, then ReduceScatter(add) over its batch quad;
each core LayerNorms its 512-row output shard.

v2 structure (vs v1):
- Phase A loads x^T / rel^T fully into SBUF once (4KB DMA rows), computes
  K+Rel projections first, then Q, then V, with 1024-token psum tiles.
- Phase B1 computes bd = rr_q . r_k for ALL 4 heads up front, streaming the
  flat rel-shift buffers (one per head) to DRAM; the shear reads for B2
  pipeline behind the writes.
- Phase B2 loops (half, head): scores = ac matmul + identity-matmul add of
  the shifted bd, exp()'d straight out of PSUM into SBUF (P natural), then
  PE-transposed per 128-column block and fed to the context matmul.
- Output projection + ReduceScatter + LayerNorm run per 1024-token half,
  overlapping the collective with the other half's attention compute.
- PSUM plan (8 banks): big pool 2x[128,1024]f32 (4) + small pool 2x2KB (2)
  + accum pool 2x[65,512]f32 (2), shared across phases.
"""

import numpy as np

B, L, D, NH, DH = 2, 2048, 1024, 16, 64
NHL = 4
P = 128
SCALE = 1.0 / np.sqrt(DH)
LN_EPS = 1e-5
N_CORES = 8

_CACHE = {}


def _build_program():
    import concourse.bacc as bacc
    import concourse.mybir as mybir
    import concourse.tile as tile
    from concourse.masks import make_identity

    F32 = mybir.dt.float32
    F16 = mybir.dt.float16
    AF = mybir.ActivationFunctionType
    AX = mybir.AxisListType
    OP = mybir.AluOpType

    nc = bacc.Bacc("TRN2", target_bir_lowering=False, debug=False,
                   num_devices=N_CORES)

    xT = nc.declare_dram_parameter("xT", [D, L], F16, isOutput=False)
    relT = nc.declare_dram_parameter("relT", [D, L], F16, isOutput=False)
    xres = nc.declare_dram_parameter("xres", [512, D], F32, isOutput=False)
    Wq = nc.declare_dram_parameter("Wq", [D, 256], F16, isOutput=False)
    Wk = nc.declare_dram_parameter("Wk", [D, 256], F16, isOutput=False)
    Wv = nc.declare_dram_parameter("Wv", [D, 256], F16, isOutput=False)
    Wrel = nc.declare_dram_parameter("Wrel", [D, 256], F16, isOutput=False)
    Wout = nc.declare_dram_parameter("Wout", [256, D], F16, isOutput=False)
    rwb = nc.declare_dram_parameter("rwb", [256], F32, isOutput=False)
    rrb = nc.declare_dram_parameter("rrb", [256], F32, isOutput=False)
    mask01 = nc.declare_dram_parameter("mask01", [L], F32, isOutput=False)
    gamma = nc.declare_dram_parameter("gamma", [D], F32, isOutput=False)
    beta = nc.declare_dram_parameter("beta", [D], F32, isOutput=False)
    out = nc.declare_dram_parameter("out", [512, D], F32, isOutput=True)

    import contextlib

    with tile.TileContext(nc) as tc:
        with contextlib.ExitStack() as _st:
            pers = _st.enter_context(tc.tile_pool(name="persist", bufs=1))
            dram = _st.enter_context(tc.tile_pool(name="dram", bufs=1, space="DRAM"))
            psBig = _st.enter_context(tc.tile_pool(name="psBig", bufs=2, space="PSUM"))
            psSmall = _st.enter_context(tc.tile_pool(name="psSmall", bufs=2, space="PSUM"))
            psAcc = _st.enter_context(tc.tile_pool(name="psAcc", bufs=2, space="PSUM"))
            ident = pers.tile([P, P], F16)
            make_identity(nc, ident[:])
            ones_r = pers.tile([1, 64], F16)
            nc.vector.memset(ones_r[:], 1.0)
            nbias = pers.tile([P, 1], F32)
            nc.vector.memset(nbias[:], -4.0)
            m01 = pers.tile([P, 16], F32)
            nc.sync.dma_start(m01[:], mask01.rearrange("(o p) -> p o", p=P))

            rwT = [pers.tile([P, L], F16, name=f"rwT{c}") for c in range(2)]
            rrT = [pers.tile([P, L], F16, name=f"rrT{c}") for c in range(2)]
            kT = [pers.tile([P, L], F16, name=f"kT{c}") for c in range(2)]
            rkT = [pers.tile([P, L], F16, name=f"rkT{c}") for c in range(2)]
            vp = [pers.tile([P, 16, DH + 1], F16, name=f"vp{h}") for h in range(NHL)]
            ctxT = [pers.tile([P, L], F16, name=f"ctxT{c}") for c in range(2)]

            # ---------- Phase A: projections ----------
            with tc.tile_pool(name="aslab", bufs=1) as a_p:
                xfull = a_p.tile([P, 8, L], F16, name="xfull")
                nc.sync.dma_start(xfull[:], xT.rearrange("(k p) n -> p k n", p=P))
                relfull = a_p.tile([P, 8, L], F16, name="relfull")
                nc.sync.dma_start(relfull[:], relT.rearrange("(k p) n -> p k n", p=P))

                wq_r = a_p.tile([P, 8, 256], F16)
                wk_r = a_p.tile([P, 8, 256], F16)
                wv_r = a_p.tile([P, 8, 256], F16)
                wl_r = a_p.tile([P, 8, 256], F16)
                for wdram, wr in ((Wk, wk_r), (Wrel, wl_r), (Wq, wq_r), (Wv, wv_r)):
                    nc.sync.dma_start(wr[:], wdram.rearrange("(k p) n -> p k n", p=P))

                rwb_sb = a_p.tile([P, 2], F32)
                nc.sync.dma_start(rwb_sb[:], rwb.rearrange("(c p) -> p c", p=P))
                rrb_sb = a_p.tile([P, 2], F32)
                nc.sync.dma_start(rrb_sb[:], rrb.rearrange("(c p) -> p c", p=P))

                # K + Rel first (B1 needs rkT fully), then Q, then V.
                def _copy(eng, dst, src):
                    if eng is nc.scalar:
                        eng.copy(dst, src)
                    else:
                        eng.tensor_copy(dst, src)
                eng_rr = [nc.scalar, nc.vector]
                for qs in range(2):
                    Q0 = 1024 * qs
                    for ti, (wr, dsts) in enumerate(((wk_r, kT), (wl_r, rkT))):
                        for cc in range(2):
                            pk = psBig.tile([P, 1024], F32, tag="big")
                            src = xfull if ti == 0 else relfull
                            for k in range(8):
                                for hh in range(2):
                                    nc.tensor.matmul(
                                        pk[:, 512 * hh:512 * hh + 512],
                                        wr[:, k, 128 * cc:128 * cc + 128],
                                        src[:, k, Q0 + 512 * hh:Q0 + 512 * hh + 512],
                                        start=(k == 0), stop=(k == 7))
                            _copy(eng_rr[(ti + cc) % 2],
                                  dsts[cc][:, Q0:Q0 + 1024], pk[:])
                for qs in range(2):
                    Q0 = 1024 * qs
                    for cc in range(2):
                        pq = psBig.tile([P, 1024], F32, tag="big")
                        for k in range(8):
                            for hh in range(2):
                                nc.tensor.matmul(
                                    pq[:, 512 * hh:512 * hh + 512],
                                    wq_r[:, k, 128 * cc:128 * cc + 128],
                                    xfull[:, k, Q0 + 512 * hh:Q0 + 512 * hh + 512],
                                    start=(k == 0), stop=(k == 7))
                        nc.vector.tensor_scalar_add(rwT[cc][:, Q0:Q0 + 1024],
                                                    pq[:], rwb_sb[:, cc:cc + 1])
                        nc.vector.tensor_scalar_add(rrT[cc][:, Q0:Q0 + 1024],
                                                    pq[:], rrb_sb[:, cc:cc + 1])
                for jo in range(16):
                    J0 = 128 * jo
                    pv = psSmall.tile([P, 256], F32, tag="small")
                    for k in range(8):
                        nc.tensor.matmul(pv[:], xfull[:, k, J0:J0 + 128],
                                         wv_r[:, k, :], start=(k == 0), stop=(k == 7))
                    for h in range(NHL):
                        nc.vector.tensor_scalar_mul(
                            vp[h][:, jo, 0:DH], pv[:, DH * h:DH * h + DH],
                            m01[:, jo:jo + 1])
                        nc.vector.tensor_copy(vp[h][:, jo, DH:DH + 1],
                                              m01[:, jo:jo + 1])

            # ---------- Phase B1: bd matmuls for all heads -> DRAM shear ----
            pf_bufs = [dram.tile([L * (L + 1)], F16, name=f"pf{i}")
                       for i in range(NHL)]
            pf2d = [pf[0:L * (L + 1)].rearrange("(r c) -> r c", c=L + 1)
                    for pf in pf_bufs]

            with contextlib.ExitStack() as _stB:
                wt_p = _stB.enter_context(tc.tile_pool(name="wt", bufs=3))
                sh_p = _stB.enter_context(tc.tile_pool(name="sh", bufs=8))
                pn_p = _stB.enter_context(tc.tile_pool(name="pnat", bufs=8))
                pt_p = _stB.enter_context(tc.tile_pool(name="pt", bufs=3))
                bc_p = _stB.enter_context(tc.tile_pool(name="bc", bufs=2))
                odd_p = _stB.enter_context(tc.tile_pool(name="oddt", bufs=2))
                den_p = _stB.enter_context(tc.tile_pool(name="den", bufs=1))
                oc_p = _stB.enter_context(tc.tile_pool(name="oc", bufs=3))
                wo_p = _stB.enter_context(tc.tile_pool(name="wo", bufs=1))
                ln_p = _stB.enter_context(tc.tile_pool(name="ln", bufs=2))
                lng_p = _stB.enter_context(tc.tile_pool(name="lng", bufs=1))
                eng_bd = [nc.vector, nc.scalar]
                for h in range(NHL):
                    cc, par = h // 2, h % 2
                    sA = slice(64 * par, 64 * par + 64)
                    for ic in range(16):
                        I0 = 128 * ic
                        wt = wt_p.tile([P, 2049], F16, tag="wt")
                        nc.vector.memset(wt[:, 0:1], 0.0)
                        for th in range(2):
                            pb2 = psBig.tile([P, 1024], F32, tag="big")
                            for tt in range(2):
                                t = 2 * th + tt
                                nc.tensor.matmul(
                                    pb2[:, 512 * tt:512 * tt + 512],
                                    rrT[cc][sA, I0:I0 + 128],
                                    rkT[cc][sA, 512 * t:512 * t + 512],
                                    start=True, stop=True)
                            _copy(eng_bd[th],
                                  wt[:, 1 + 1024 * th:1 + 1024 * th + 1024],
                                  pb2[:])
                        nc.gpsimd.dma_start(pf2d[h][I0:I0 + 128, :], wt[:])

                # out-proj weights (used per half)
                wo_r = [wo_p.tile([P, 2, 512], F16, name=f"wo{c}") for c in range(2)]
                for c in range(2):
                    nc.sync.dma_start(
                        wo_r[c][:], Wout[128 * c:128 * c + 128, :]
                        .rearrange("p (t n) -> p t n", t=2))
                gb = lng_p.tile([P, D], F32)
                nc.gpsimd.dma_start(gb[:], gamma.ap().rearrange(
                    "(o d) -> o d", o=1).to_broadcast((P, D)))
                bb = lng_p.tile([P, D], F32)
                nc.gpsimd.dma_start(bb[:], beta.ap().rearrange(
                    "(o d) -> o d", o=1).to_broadcast((P, D)))

                attn_d = dram.tile([L, D], F16)
                rs_d = dram.tile([512, D], F16)

                # ---------- Phase B2 + C interleaved ----------
                for half in range(2):
                    H0 = 1024 * half
                    for h in range(NHL):
                        cc, par = h // 2, h % 2
                        sA = slice(64 * par, 64 * par + 64)
                        pf = pf_bufs[h][:]

                        shs = []
                        for icc in range(8):
                            I0 = H0 + 128 * icc
                            sh16 = sh_p.tile([P, L], F16, tag="sh")
                            nc.sync.dma_start(
                                sh16[:],
                                pf[L + I0 * L: L + (I0 + 128) * L]
                                .rearrange("(r c) -> r c", c=L))
                            shs.append(sh16)

                        # scores + exp -> P natural (fp16)
                        pnats = []
                        for icc in range(8):
                            I0 = H0 + 128 * icc
                            pn = pn_p.tile([P, L], F16, tag="pn")
                            for th in range(2):
                                ps = psBig.tile([P, 1024], F32, tag="big")
                                for tt in range(2):
                                    t = 2 * th + tt
                                    nc.tensor.matmul(
                                        ps[:, 512 * tt:512 * tt + 512],
                                        rwT[cc][sA, I0:I0 + 128],
                                        kT[cc][sA, 512 * t:512 * t + 512],
                                        start=True, stop=False)
                                    nc.tensor.matmul(
                                        ps[:, 512 * tt:512 * tt + 512],
                                        ident[:],
                                        shs[icc][:, 512 * t:512 * t + 512],
                                        start=False, stop=True)
                                nc.scalar.activation(
                                    pn[:, 1024 * th:1024 * th + 1024], ps[:],
                                    AF.Exp, bias=nbias[:])
                            pnats.append(pn)

                        # transpose P + context matmul
                        ot = (odd_p.tile([64, 1024], F16, tag="odd", name="ot")
                              if par == 1 else None)
                        pc0 = psAcc.tile([65, 512], F32, tag="acc")
                        pc1 = psAcc.tile([65, 512], F32, tag="acc")
                        # software-pipelined: ctx matmul for J-1 issues while
                        # the transposes for J run, hiding the DVE copy.
                        pt_prev = None
                        for J in range(16):
                            ptp = psSmall.tile([P, 1024], F16, tag="small")
                            for icc in range(8):
                                nc.tensor.matmul(
                                    ptp[:, 128 * icc:128 * icc + 128],
                                    pnats[icc][:, 128 * J:128 * J + 128],
                                    ident[:], is_transpose=True,
                                    start=True, stop=True)
                            pt_sb = pt_p.tile([P, 1024], F16, tag="pt")
                            nc.vector.tensor_copy(pt_sb[:], ptp[:])
                            if pt_prev is not None:
                                for ii, pc in enumerate((pc0, pc1)):
                                    nc.tensor.matmul(
                                        pc[:], vp[h][:, J - 1, :],
                                        pt_prev[:, 512 * ii:512 * ii + 512],
                                        start=(J == 1), stop=False)
                            pt_prev = pt_sb
                        for ii, pc in enumerate((pc0, pc1)):
                            nc.tensor.matmul(
                                pc[:], vp[h][:, 15, :],
                                pt_prev[:, 512 * ii:512 * ii + 512],
                                start=False, stop=True)

                        # softmax denominator + normalize
                        den_sb = den_p.tile([P, 1024], F32, tag="den_sb",
                                            name="den_sb")
                        den0 = den_p.tile([1, 1024], F32, tag="den0", name="den0")
                        rec0 = den_p.tile([1, 1024], F32, tag="rec0", name="rec0")
                        recr = den_p.tile([1, 1024], F16, tag="recr", name="recr")
                        scr = den_p.tile([1, 1024], F32, tag="scr", name="scr")
                        nc.vector.tensor_copy(den_sb[64:65, 0:512], pc0[64:65, :])
                        nc.vector.tensor_copy(den_sb[64:65, 512:1024],
                                              pc1[64:65, :])
                        nc.sync.dma_start(den0[0:1, :], den_sb[64:65, 0:1024])
                        nc.vector.reciprocal_approx_accurate(
                            rec0[0:1, :], den0[0:1, :], scr[0:1, :])
                        nc.vector.tensor_copy(recr[0:1, :], rec0[0:1, :])
                        for ii, pc in enumerate((pc0, pc1)):
                            i0 = H0 + 512 * ii
                            pb = psSmall.tile([64, 512], F32, tag="small")
                            nc.tensor.matmul(pb[:], ones_r[0:1, :],
                                             recr[0:1, 512 * ii:512 * ii + 512],
                                             start=True, stop=True)
                            bcf = bc_p.tile([64, 512], F32, tag="bc")
                            nc.scalar.copy(bcf[:], pb[:])
                            if par == 0:
                                nc.vector.tensor_mul(ctxT[cc][0:64, i0:i0 + 512],
                                                     pc[0:64, :], bcf[:])
                            else:
                                nc.vector.tensor_mul(ot[:, 512 * ii:512 * ii + 512],
                                                     pc[0:64, :], bcf[:])
                        if par == 1:
                            nc.sync.dma_start(ctxT[cc][64:128, H0:H0 + 1024], ot[:])

                    # ---- Phase C for this half: out-proj + RS + LN ----
                    for r in (2 * half, 2 * half + 1):
                        for ic4 in range(4):
                            I0 = 512 * r + 128 * ic4
                            for t in range(2):
                                po = psSmall.tile([P, 512], F32, tag="small")
                                for c in range(2):
                                    nc.tensor.matmul(po[:],
                                                     ctxT[c][:, I0:I0 + 128],
                                                     wo_r[c][:, t, :],
                                                     start=(c == 0), stop=(c == 1))
                                ao = oc_p.tile([P, 512], F16, tag="ao")
                                if t == 0:
                                    nc.vector.tensor_copy(ao[:], po[:])
                                else:
                                    nc.scalar.copy(ao[:], po[:])
                                nc.sync.dma_start(
                                    attn_d[I0:I0 + 128, 512 * t:512 * t + 512],
                                    ao[:])
                        nc.gpsimd.collective_compute(
                            "ReduceScatter", OP.add,
                            replica_groups=[[0, 1, 2, 3], [4, 5, 6, 7]],
                            ins=[attn_d[512 * r:512 * r + 512, :].opt()],
                            outs=[rs_d[128 * r:128 * r + 128, :].opt()],
                        )
                    for r in (2 * half, 2 * half + 1):
                        R0 = 128 * r
                        zt16 = ln_p.tile([P, D], F16, tag="zt16")
                        nc.sync.dma_start(zt16[:], rs_d[R0:R0 + 128, :])
                        zt = ln_p.tile([P, D], F32, tag="zt")
                        nc.gpsimd.tensor_copy(zt[:], zt16[:])
                        xr = ln_p.tile([P, D], F32, tag="xr")
                        nc.sync.dma_start(xr[:], xres[R0:R0 + 128, :])
                        nc.gpsimd.tensor_add(zt[:], zt[:], xr[:])
                        mu = ln_p.tile([P, 1], F32, tag="mu")
                        nc.vector.tensor_reduce(mu[:], zt[:], AX.X, OP.add)
                        nc.gpsimd.tensor_scalar_mul(mu[:], mu[:], 1.0 / D)
                        nc.gpsimd.tensor_scalar_sub(zt[:], zt[:], mu[:])
                        nc.gpsimd.tensor_mul(xr[:], zt[:], zt[:])
                        var = ln_p.tile([P, 1], F32, tag="var")
                        nc.vector.tensor_reduce(var[:], xr[:], AX.X, OP.add)
                        nc.gpsimd.tensor_scalar_mul(var[:], var[:], 1.0 / D)
                        nc.gpsimd.tensor_scalar_add(var[:], var[:], LN_EPS)
                        sd = ln_p.tile([P, 1], F32, tag="sd")
                        nc.scalar.activation(sd[:], var[:], AF.Sqrt)
                        isd = ln_p.tile([P, 1], F32, tag="isd")
                        nc.vector.reciprocal(isd[:], sd[:])
                        nc.gpsimd.tensor_scalar_mul(zt[:], zt[:], isd[:])
                        nc.gpsimd.tensor_mul(zt[:], zt[:], gb[:])
                        nc.gpsimd.tensor_add(zt[:], zt[:], bb[:])
                        nc.sync.dma_start(out[R0:R0 + 128, :], zt[:])

    nc.compile()
    return nc


def _prep_inputs(x, relative_pos, r_w_bias, r_r_bias, attn_mask,
                 W_qkv, W_rel, W_out, ln_gamma, ln_beta):
    in_maps = []
    relT = np.ascontiguousarray(relative_pos.T).astype(np.float16)
    m01f = (~np.asarray(attn_mask).astype(bool)).astype(np.float32)
    for c in range(N_CORES):
        b, g = c // 4, c % 4
        h0 = 4 * g
        cols = slice(DH * h0, DH * h0 + 256)
        im = dict(
            xT=np.ascontiguousarray(x[b].T).astype(np.float16),
            relT=relT,
            xres=np.ascontiguousarray(np.concatenate(
                [x[b, 512 * r + 128 * g: 512 * r + 128 * g + 128, :]
                 for r in range(4)], axis=0)).astype(np.float32),
            Wq=np.ascontiguousarray(
                W_qkv[:, DH * h0:DH * h0 + 256] * SCALE).astype(np.float16),
            Wk=np.ascontiguousarray(
                W_qkv[:, D + DH * h0: D + DH * h0 + 256]).astype(np.float16),
            Wv=np.ascontiguousarray(
                W_qkv[:, 2 * D + DH * h0: 2 * D + DH * h0 + 256]).astype(np.float16),
            Wrel=np.ascontiguousarray(W_rel[:, cols]).astype(np.float16),
            Wout=np.ascontiguousarray(W_out[cols, :]).astype(np.float16),
            rwb=np.ascontiguousarray(
                r_w_bias[h0:h0 + 4].reshape(-1) * SCALE).astype(np.float32),
            rrb=np.ascontiguousarray(
                r_r_bias[h0:h0 + 4].reshape(-1) * SCALE).astype(np.float32),
            mask01=m01f[b],
            gamma=np.asarray(ln_gamma).astype(np.float32),
            beta=np.asarray(ln_beta).astype(np.float32),
        )
        in_maps.append(im)
    return in_maps


def kernel(**inputs):
    from concourse.bass_utils import run_bass_kernel_spmd

    if "nc" not in _CACHE:
        _CACHE["nc"] = _build_program()
    nc = _CACHE["nc"]

    in_maps = _prep_inputs(**{k: np.asarray(v) for k, v in inputs.items()})
    res = run_bass_kernel_spmd(nc, in_maps, list(range(N_CORES)))
    outp = np.empty((B, L, D), np.float32)
    for c in range(N_CORES):
        b, g = c // 4, c % 4
        o = res.results[c]["out"]
        for r in range(4):
            outp[b, 512 * r + 128 * g: 512 * r + 128 * g + 128, :] = \
                o[128 * r:128 * r + 128, :]
    return outp
